# revision 1
# baseline (speedup 1.0000x reference)
# Multi-head attention (B=2, S=2048, D=1024, H=16, head_dim=64) with bool mask,
# sharded across 8 TRN2 NeuronCores: core c -> batch c//4, heads 4*(c%4)..4*(c%4)+3.
#
# Per-core device kernel (scores computed transposed: scoresT[k, q]):
#   scoresT = K @ Q^T                (PE bf16, lhsT = K^T chunk, rhs = Q^T)
#   attnT   = exp(scoresT/8) * (1-m)T (ACT exp with scale=1/8 -> bf16, DVE mult)
#   outT'   = [V | 1]^T @ attnT      (PE bf16; row 64 = softmax denominator Z)
#   out     = transpose(outT') / Z   (PE transpose + batched DVE normalize)
#
# Host side (inside kernel()): slice per-core shards, pre-transpose Q/K per head
# ([64, S] head-dim-major, bf16), pre-transpose the inverted mask to bf16,
# reassemble the 8 per-core bf16 outputs into the full f32 [B, S, D] output.

import sys

import numpy as np

for _p in ("/opt/trn_rl_repo",):
    if _p not in sys.path:
        sys.path.insert(0, _p)

import ml_dtypes

import concourse.bass as bass  # noqa: F401  (engine types reachable via nc)
import concourse.tile as tile
from concourse import bacc, mybir
from concourse.bass_utils import run_bass_kernel_spmd
from concourse.masks import make_identity

F32 = mybir.dt.float32
F32R = mybir.dt.float32r
BF16 = mybir.dt.bfloat16

S = 2048          # sequence length
HD = 64           # head dim
HPC = 4           # heads per core
NCORES = 8
B = 2
H = 16
D = H * HD


def build_program(s=S, act_dtype=BF16, qk_dtype=BF16, n_psS=2, reps=1):
    """Build the single-core SPMD program. Returns the compiled Bacc object.

    reps>1 emits the whole body (loads+compute+stores) that many times in one
    NEFF — used to measure device time by wall-clock differencing."""
    nc = bacc.Bacc()

    KS = s // 128            # number of k strips
    QG = 1024 if s >= 1024 else s   # q group width (ACT/DVE instruction width)
    NQG = s // QG            # q groups
    NQC = max(QG // 512, 1)  # 512-wide matmul chunks per q group
    QC = min(512, QG)        # matmul chunk width
    JT = QG // 128           # out-transpose chunks per q group

    qkT_d = nc.declare_dram_parameter("qkT", [2, HPC * HD, s], qk_dtype, isOutput=False)
    v_d = nc.declare_dram_parameter("v", [s, HPC * HD], BF16, isOutput=False)
    nmT_d = nc.declare_dram_parameter("nmT", [s, s], BF16, isOutput=False)
    out_d = nc.declare_dram_parameter("out", [s, HPC * HD], BF16, isOutput=True)

    # DRAM views with the k/q axis split into strips of 128 partitions
    nm_view = nmT_d[:].rearrange("(ks p) q -> p ks q", p=128)
    v_view = v_d[:].rearrange("(ks p) c -> p ks c", p=128)
    out_view = out_d[:].rearrange("(sq p) c -> p sq c", p=128)

    with tile.TileContext(nc) as tc:
        with (
            tc.tile_pool(name="const", bufs=1) as const,
            tc.tile_pool(name="wq", bufs=1) as wq,
            tc.tile_pool(name="vstg", bufs=1) as vstg,
            tc.tile_pool(name="attn", bufs=20) as apool,
            tc.tile_pool(name="fin", bufs=2) as fpool,
            tc.tile_pool(name="stat", bufs=4) as spool,
            tc.tile_pool(name="oasm", bufs=1) as opool,
            tc.tile_pool(name="psS", bufs=n_psS, space="PSUM") as psS_pool,
            # psO (AV accumulator, [65,QG]=2 banks) and pn (out-transpose
            # target, [128,JT,128]=2 banks) share one tag with bufs=2: the
            # two slots alternate psO/pn roles, so AV of group g only waits
            # for the finalize reads of group g-2 (1.5 groups of slack).
            tc.tile_pool(name="psF", bufs=2, space="PSUM") as psF_pool,
        ):
            ident = const.tile([128, 128], F32)
            make_identity(nc, ident)

            # Preload the exp table (emitted before any real exp; runs while
            # the first DMAs stream).
            warm = const.tile([128, 1], F32)
            nc.vector.memset(warm, 0.0)
            nc.scalar.activation(warm, warm, mybir.ActivationFunctionType.Exp)

            # Warm the PE HAM clock gate while input DMAs run: ~3us of dummy
            # matmuls (transpose-mode doesn't count as PE-busy for HAM) so
            # the first real QKs run at 2.4GHz.
            zb = const.tile([128, 128], BF16)
            nc.vector.memset(zb, 0.0)
            for _ in range(24):
                wmm = psS_pool.tile([128, QG], F32, tag="psS")
                nc.tensor.matmul(
                    wmm[:, :128], lhsT=zb[0:64, :], rhs=zb[0:64, :],
                    start=True, stop=True,
                )

            def qk_src(pair):
                return qkT_d[:, 128 * pair:128 * pair + 128, :].rearrange(
                    "t p s -> p t s"
                )

            def emit_body():
                # Q^T / K^T head pairs: [128, s] (head 2p on partitions 0-63,
                # head 2p+1 on partitions 64-127). The first pair's q and k
                # halves ride different HWDGE queues in parallel; everything
                # else is emitted in the order compute consumes it.
                qks = []
                for pair in range(HPC // 2):
                    qk = wq.tile([128, 2, s], qk_dtype, tag=f"qkT{pair}")
                    qks.append(qk)
                v_sb = vstg.tile([128, KS, HPC * HD], BF16)
                nm_sb = wq.tile([128, KS, s], BF16, tag="nm")
                KH = KS // 2
                nc.scalar.dma_start(out=qks[0][:, 0, :], in_=qk_src(0)[:, 0, :])
                nc.sync.dma_start(out=qks[0][:, 1, :], in_=qk_src(0)[:, 1, :])
                nc.sync.dma_start(out=v_sb[:, :KH], in_=v_view[:, :KH])
                nc.sync.dma_start(out=v_sb[:, KH:], in_=v_view[:, KH:])
                for pair in range(1, HPC // 2):
                    nc.scalar.dma_start(out=qks[pair], in_=qk_src(pair))
                for ks in range(KS):
                    nc.sync.dma_start(out=nm_sb[:, ks, :], in_=nm_view[:, ks, :])

                # V' = [V | 1] per head, bf16; cast in halves so early AVs
                # only wait on the first half of the V DMA.
                vps = []
                for h in range(HPC):
                    vp = wq.tile([128, KS, HD + 1], BF16, tag=f"vp{h}")
                    vps.append(vp)
                for half in range(2):
                    ksl = slice(half * KH, KH + half * KH)
                    for h in range(HPC):
                        nc.vector.tensor_copy(
                            out=vps[h][:, ksl, 0:HD],
                            in_=v_sb[:, ksl, h * HD:(h + 1) * HD],
                        )
                        nc.vector.memset(vps[h][:, ksl, HD:HD + 1], 1.0)

                out_asm = opool.tile([128, KS, HPC * HD], BF16)

                # Emission state threading three overlapped group pipelines:
                #   carry - group awaiting last AV (stop=True) + psO->oT copy
                #   fin   - group awaiting its transpose+normalize steps
                fin = {"pend": None, "idx": 0, "pn": None}
                N_FIN = JT + 1  # JT transposes + one batched normalize step

                def finalize_step():
                    """One finalize chunk of a finished q-group: steps
                    0..JT-1 transpose [65,128] pieces into pn; step JT does
                    one strided reciprocal over the JT Z values and two
                    broadcast multiplies (batched - avoids per-strip
                    sequencer overhead)."""
                    h, qg, oT = fin["pend"]
                    if fin["idx"] >= N_FIN:
                        return
                    j = fin["idx"]
                    fin["idx"] += 1
                    if j == 0:
                        pn_t = psF_pool.tile([128, JT, 128], F32, tag="fin")
                        fin["pn"] = pn_t
                    pn = fin["pn"]
                    if j < JT:
                        nc.tensor.transpose(
                            pn[:, j, :HD + 1],
                            oT[:, j * 128:(j + 1) * 128],
                            ident[:HD + 1, :HD + 1],
                        )
                        return
                    rec8 = spool.tile([128, JT], F32)
                    nc.vector.reciprocal(rec8, pn[:, :, HD])
                    half = (JT + 1) // 2
                    for lo in range(0, JT, half):
                        hi = min(lo + half, JT)
                        sq0 = qg * JT + lo
                        nc.vector.tensor_mul(
                            out_asm[:, sq0:sq0 + hi - lo, h * HD:(h + 1) * HD],
                            pn[:, lo:hi, 0:HD],
                            rec8[:, lo:hi].to_broadcast([128, hi - lo, HD]),
                        )
                        if h == HPC - 1:
                            eng = nc.sync if lo == 0 else nc.scalar
                            eng.dma_start(
                                out=out_view[:, sq0:sq0 + hi - lo, :],
                                in_=out_asm[:, sq0:sq0 + hi - lo, :],
                            )

                def emit_carry(carry):
                    """Last AV (stop=True) + psO->SBUF copy of a group."""
                    ch, cqg, cpsO, cat = carry
                    for qc in range(NQC):
                        nc.tensor.matmul(
                            cpsO[:, qc * QC:(qc + 1) * QC],
                            lhsT=vps[ch][:, KS - 1, :],
                            rhs=cat[:, qc * QC:(qc + 1) * QC],
                            start=(KS == 1),
                            stop=True,
                        )
                    oT = fpool.tile([HD + 1, QG], F32, tag="oT")
                    nc.vector.tensor_copy(oT, cpsO)
                    # flush unfinished finalize steps of the older group
                    while fin["pend"] is not None and fin["idx"] < N_FIN:
                        finalize_step()
                    fin["pend"] = (ch, cqg, oT)
                    fin["idx"] = 0

                carry = None
                groups = [(h, qg) for h in range(HPC) for qg in range(NQG)]
                for h, qg in groups:
                    base = 64 * (h % 2)
                    qt_r = qks[h // 2][:, 0, :]
                    kt_r = qks[h // 2][:, 1, :]
                    q0 = qg * QG
                    psO = None
                    at_prev = None
                    for ks in range(KS):
                        # AV one strip behind QK, emitted BEFORE this strip's
                        # QK so it isn't queued behind QK's psum-slot wait.
                        if at_prev is not None:
                            if psO is None:
                                psO = psF_pool.tile(
                                    [HD + 1, QG], F32, tag="fin"
                                )
                            for qc in range(NQC):
                                nc.tensor.matmul(
                                    psO[:, qc * QC:(qc + 1) * QC],
                                    lhsT=vps[h][:, ks - 1, :],
                                    rhs=at_prev[:, qc * QC:(qc + 1) * QC],
                                    start=(ks == 1),
                                    stop=False,
                                )
                        # Transpose+normalize of an older group, interleaved
                        # so it never stalls the PE pipeline.
                        if fin["pend"] is not None and ks >= 1:
                            finalize_step()
                        psS = psS_pool.tile([128, QG], F32)
                        for qc in range(NQC):
                            nc.tensor.matmul(
                                psS[:, qc * QC:(qc + 1) * QC],
                                lhsT=kt_r[base:base + HD, ks * 128:(ks + 1) * 128],
                                rhs=qt_r[base:base + HD,
                                         q0 + qc * QC:q0 + (qc + 1) * QC],
                                start=True,
                                stop=True,
                            )
                        if ks == 0 and carry is not None:
                            emit_carry(carry)
                            carry = None
                        at = apool.tile([128, QG], act_dtype, tag="at")
                        nc.scalar.activation(
                            at, psS, mybir.ActivationFunctionType.Exp,
                            scale=0.125,
                        )
                        nc.vector.tensor_mul(at, at, nm_sb[:, ks, q0:q0 + QG])
                        at_prev = at
                    carry = (h, qg, psO, at_prev)
                emit_carry(carry)
                while fin["idx"] < N_FIN:
                    finalize_step()

            for _ in range(reps):
                emit_body()
    nc.compile()
    return nc


_CACHE = {}


def _get_nc():
    if "nc" not in _CACHE:
        _CACHE["nc"] = build_program()
    return _CACHE["nc"]


def make_in_maps(q, k, v, mask, s=S):
    """Shard full inputs into 8 per-core input maps (host-side layout prep)."""
    q = np.asarray(q, dtype=np.float32)
    k = np.asarray(k, dtype=np.float32)
    v = np.asarray(v, dtype=np.float32)
    mask = np.asarray(mask)
    nh = q.shape[-1] // HD
    in_maps = []
    for c in range(NCORES):
        b, g = divmod(c, NCORES // B)
        h0 = HPC * g
        qs = q[b].reshape(s, nh, HD)[:, h0:h0 + HPC, :]      # [s, HPC, 64]
        ks_ = k[b].reshape(s, nh, HD)[:, h0:h0 + HPC, :]
        qkT = np.empty((2, HPC * HD, s), ml_dtypes.bfloat16)
        qkT[0] = qs.transpose(1, 2, 0).reshape(HPC * HD, s)
        qkT[1] = ks_.transpose(1, 2, 0).reshape(HPC * HD, s)
        vc = np.ascontiguousarray(v[b, :, h0 * HD:(h0 + HPC) * HD]).astype(
            ml_dtypes.bfloat16
        )
        nmT = np.ascontiguousarray((~mask[b]).T).astype(ml_dtypes.bfloat16)
        in_maps.append({"qkT": qkT, "v": vc, "nmT": nmT})
    return in_maps


def assemble_out(results, s=S, d=D):
    out = np.empty((B, s, d), np.float32)
    for c in range(NCORES):
        b, g = divmod(c, NCORES // B)
        out[b, :, g * HPC * HD:(g + 1) * HPC * HD] = results[c]["out"]
    return out


def kernel(q, k, v, mask):
    nc = _get_nc()
    in_maps = make_in_maps(q, k, v, mask)
    res = run_bass_kernel_spmd(nc, in_maps, list(range(NCORES))).results
    return assemble_out(res)



# revision 9
# speedup vs baseline: 1.0088x; 1.0088x over previous
# Multi-head attention (B=2, S=2048, D=1024, H=16, head_dim=64) with bool mask,
# sharded across 8 TRN2 NeuronCores: core c -> batch c//4, heads 4*(c%4)..4*(c%4)+3.
#
# Per-core device kernel (scores computed transposed: scoresT[k, q]):
#   scoresT = K @ Q^T                 (PE bf16, lhsT = K^T strip, rhs = Q^T)
#   atp     = exp(scoresT/8)          (ACT exp scale=1/8, psum -> psum bf16)
#   at      = atp * (1-m)T            (DVE mult, psum -> SBUF bf16)
#   out[q,d] += at_chunk^T @ [V|1]    (PE bf16: lhsT = at chunk (stationary),
#                                      rhs = V'[128,65]; col 64 accumulates Z)
#   out     = psO[:, :, 0:64] / Z     (DVE reciprocal + broadcast multiply)
#
# The AV matmul uses the attention chunk as the stationary operand so the
# output lands non-transposed ([q, d] with q on partitions): free size is 65
# instead of 512 per instruction (half the PE cycles of the V-stationary
# form) and the final PE transposes disappear entirely.
#
# Host side (inside kernel()): slice per-core shards, pre-transpose Q/K per
# head ([64, S] head-dim-major, bf16), pre-transpose the inverted mask to
# bf16, reassemble the 8 per-core bf16 outputs into the full f32 output.

import sys

import numpy as np

for _p in ("/opt/trn_rl_repo",):
    if _p not in sys.path:
        sys.path.insert(0, _p)

import ml_dtypes

import concourse.bass as bass  # noqa: F401  (engine types reachable via nc)
import concourse.tile as tile
from concourse import bacc, mybir
from concourse.bass_utils import run_bass_kernel_spmd

F32 = mybir.dt.float32
BF16 = mybir.dt.bfloat16

S = 2048          # sequence length
HD = 64           # head dim
HPC = 4           # heads per core
NCORES = 8
B = 2
H = 16
D = H * HD


def build_program(s=S, reps=1):
    """Build the single-core SPMD program. Returns the compiled Bacc object.

    reps>1 emits the whole body (loads+compute+stores) that many times in one
    NEFF — used to measure device time by wall-clock differencing."""
    nc = bacc.Bacc()

    KS = s // 128            # number of k strips
    QG = 1024 if s >= 1024 else s   # q group width (ACT/DVE instruction width)
    NQG = s // QG            # q groups
    NQC = max(QG // 512, 1)  # 512-wide matmul chunks per q group (psum bank)
    QC = min(512, QG)        # matmul chunk width
    NCH = QG // 128          # 128-wide q chunks per group (AV granularity)
    CPB = 4                  # psO chunks per 2KB psum bank (zero region)

    qkT_d = nc.declare_dram_parameter("qkT", [2, HPC * HD, s], BF16, isOutput=False)
    v_d = nc.declare_dram_parameter("v", [s, HPC * HD], BF16, isOutput=False)
    nmT_d = nc.declare_dram_parameter("nmT", [s, s], BF16, isOutput=False)
    out_d = nc.declare_dram_parameter("out", [s, HPC * HD], BF16, isOutput=True)

    # DRAM views with the k/q axis split into strips of 128 partitions
    nm_view = nmT_d[:].rearrange("(ks p) q -> p ks q", p=128)
    v_view = v_d[:].rearrange("(ks p) (h d) -> p ks h d", p=128, h=HPC)
    out_view = out_d[:].rearrange("(sq p) c -> p sq c", p=128)

    with tile.TileContext(nc) as tc:
        with (
            tc.tile_pool(name="const", bufs=1) as const,
            tc.tile_pool(name="wq", bufs=1) as wq,
            tc.tile_pool(name="attn", bufs=20) as apool,
            tc.tile_pool(name="stat", bufs=4) as spool,
            tc.tile_pool(name="oasm", bufs=1) as opool,
            tc.tile_pool(name="psS", bufs=2, space="PSUM") as psS_pool,
            tc.tile_pool(name="psO", bufs=2, space="PSUM") as psO_pool,
        ):
            # Preload the exp table (emitted before any real exp; runs while
            # the first DMAs stream).
            warm = const.tile([128, 1], F32)
            nc.vector.memset(warm, 0.0)
            nc.scalar.activation(warm, warm, mybir.ActivationFunctionType.Exp)

            # Warm the PE HAM clock gate while input DMAs run: ~3us of dummy
            # matmuls so the first real QKs run at 2.4GHz.
            zb = const.tile([128, 128], BF16)
            nc.vector.memset(zb, 0.0)
            for _ in range(24):
                wmm = psS_pool.tile([128, QG], F32, tag="psS")
                nc.tensor.matmul(
                    wmm[:, :128], lhsT=zb[0:64, :], rhs=zb[0:64, :],
                    start=True, stop=True,
                )

            def qk_src(pair):
                return qkT_d[:, 128 * pair:128 * pair + 128, :].rearrange(
                    "t p s -> p t s"
                )

            def emit_body():
                # Q^T / K^T head pairs: [128, s] (head 2p on partitions 0-63,
                # head 2p+1 on partitions 64-127).
                qks = []
                for pair in range(HPC // 2):
                    qk = wq.tile([128, 2, s], BF16, tag=f"qkT{pair}")
                    qks.append(qk)
                # V' staging: [128, ks, h, 65] with a ones column at 64 so the
                # AV matmul's 65th output column accumulates the softmax
                # denominator Z. V lands via interleaved DMA; the ones column
                # is memset once (disjoint subtile, no dependency on the DMA).
                vps = wq.tile([128, KS, HPC, HD + 1], BF16, tag="vps")
                nm_sb = wq.tile([128, KS, s], BF16, tag="nm")
                nc.vector.memset(vps[:, :, :, HD:HD + 1], 1.0)

                # DMA choreography (s=2048): two queues only — SP (nc.sync)
                # and Pool SWDGE (nc.gpsimd) — so the ACT and DVE sequencers
                # are never blocked behind a DMA wait. Ordered by first use:
                # K strips + first Q group first, mask halves interleaved,
                # V early (AV matmuls sit in the in-order PE queue).
                QH = QG  # nm half width
                if s == 2048:
                    A, Bq = nc.sync, nc.gpsimd
                    # A queue
                    A.dma_start(out=qks[0][:, 1, 0:512], in_=qk_src(0)[:, 1, 0:512])
                    A.dma_start(out=qks[0][:, 0, 0:QG], in_=qk_src(0)[:, 0, 0:QG])
                    A.dma_start(out=qks[0][:, 1, 512:1024], in_=qk_src(0)[:, 1, 512:1024])
                    # B queue: V first (PE's AV instructions wait on it
                    # in-order; per-head DMAs keep the balanced AP <= 3 dims),
                    # interleaved with odd mask halves of q-group 0.
                    Bq.dma_start(out=vps[:, :, 0, 0:HD], in_=v_view[:, :, 0])
                    Bq.dma_start(out=nm_sb[:, 1, 0:QH], in_=nm_view[:, 1, 0:QH])
                    Bq.dma_start(out=vps[:, :, 1, 0:HD], in_=v_view[:, :, 1])
                    Bq.dma_start(out=nm_sb[:, 3, 0:QH], in_=nm_view[:, 3, 0:QH])
                    Bq.dma_start(out=vps[:, :, 2, 0:HD], in_=v_view[:, :, 2])
                    Bq.dma_start(out=nm_sb[:, 5, 0:QH], in_=nm_view[:, 5, 0:QH])
                    Bq.dma_start(out=vps[:, :, 3, 0:HD], in_=v_view[:, :, 3])
                    for ks in range(7, KS, 2):
                        Bq.dma_start(out=nm_sb[:, ks, 0:QH], in_=nm_view[:, ks, 0:QH])
                    # A queue continued: even mask halves + K/Q remainder
                    A.dma_start(out=nm_sb[:, 0, 0:QH], in_=nm_view[:, 0, 0:QH])
                    A.dma_start(out=nm_sb[:, 2, 0:QH], in_=nm_view[:, 2, 0:QH])
                    A.dma_start(out=qks[0][:, 1, 1024:2048], in_=qk_src(0)[:, 1, 1024:2048])
                    for ks in range(4, KS, 2):
                        A.dma_start(out=nm_sb[:, ks, 0:QH], in_=nm_view[:, ks, 0:QH])
                    A.dma_start(out=qks[0][:, 0, QG:2 * QG], in_=qk_src(0)[:, 0, QG:2 * QG])
                    A.dma_start(out=qks[1], in_=qk_src(1))
                    # q-group 1 mask halves, split across both queues
                    for ks in range(0, KS, 2):
                        A.dma_start(out=nm_sb[:, ks, QH:2 * QH], in_=nm_view[:, ks, QH:2 * QH])
                    for ks in range(1, KS, 2):
                        Bq.dma_start(out=nm_sb[:, ks, QH:2 * QH], in_=nm_view[:, ks, QH:2 * QH])
                else:
                    A, Bq = nc.sync, nc.gpsimd
                    A.dma_start(out=qks[0], in_=qk_src(0))
                    for hh in range(HPC):
                        Bq.dma_start(out=vps[:, :, hh, 0:HD], in_=v_view[:, :, hh])
                    for pair in range(1, HPC // 2):
                        A.dma_start(out=qks[pair], in_=qk_src(pair))
                    for ks in range(KS):
                        (A if ks % 2 == 0 else Bq).dma_start(
                            out=nm_sb[:, ks, :], in_=nm_view[:, ks, :]
                        )

                out_asm = opool.tile([128, KS, HPC * HD], BF16)

                def emit_carry(carry):
                    """Last two AV strips (lag-2 emission) + finalize: Z
                    reciprocal, broadcast normalize, and the output DMA once
                    the last head of a q-group completes."""
                    ch, cqg, cpsO, at_tail = carry
                    for i, (cat, cks) in enumerate(at_tail):
                        last = i == len(at_tail) - 1
                        for j in range(NCH):
                            nc.tensor.matmul(
                                cpsO[:, j, 0:HD + 1],
                                lhsT=cat[:, j * 128:(j + 1) * 128],
                                rhs=vps[:, cks, ch, :],
                                # start/stop once per psum bank (4 chunks of
                                # 512B share a 2KB zero region)
                                start=(cks == 0 and j % CPB == 0),
                                stop=last and (j % CPB == CPB - 1 or j == NCH - 1),
                            )
                    rec = spool.tile([128, NCH], F32)
                    nc.vector.reciprocal(rec, cpsO[:, :, HD])
                    half = max(NCH // 2, 1)
                    for lo in range(0, NCH, half):
                        hi = min(lo + half, NCH)
                        sq0 = cqg * NCH + lo
                        nc.vector.tensor_mul(
                            out_asm[:, sq0:sq0 + hi - lo, ch * HD:(ch + 1) * HD],
                            cpsO[:, lo:hi, 0:HD],
                            rec[:, lo:hi].to_broadcast([128, hi - lo, HD]),
                        )
                        if ch == HPC - 1:
                            eng = nc.sync if lo == 0 else nc.gpsimd
                            eng.dma_start(
                                out=out_view[:, sq0:sq0 + hi - lo, :],
                                in_=out_asm[:, sq0:sq0 + hi - lo, :],
                            )

                carry = None
                groups = [(h, qg) for h in range(HPC) for qg in range(NQG)]
                for h, qg in groups:
                    base = 64 * (h % 2)
                    qt_r = qks[h // 2][:, 0, :]
                    kt_r = qks[h // 2][:, 1, :]
                    q0 = qg * QG
                    psO = None
                    ats = {}
                    for ks in range(KS):
                        # AV emitted with a lag of 2 strips so the psO
                        # allocation (single-buffered: 2 psum banks) never
                        # blocks the in-order PE queue on the previous
                        # group's normalize reads.
                        if ks - 2 in ats:
                            at2 = ats.pop(ks - 2)
                            if psO is None:
                                psO = psO_pool.tile([128, NCH, 128], F32)
                            for j in range(NCH):
                                nc.tensor.matmul(
                                    psO[:, j, 0:HD + 1],
                                    lhsT=at2[:, j * 128:(j + 1) * 128],
                                    rhs=vps[:, ks - 2, h, :],
                                    start=(ks == 2 and j % CPB == 0),
                                    stop=False,
                                )
                        psS = psS_pool.tile([128, QG], F32, tag="psS")
                        for qc in range(NQC):
                            nc.tensor.matmul(
                                psS[:, qc * QC:(qc + 1) * QC],
                                lhsT=kt_r[base:base + HD, ks * 128:(ks + 1) * 128],
                                rhs=qt_r[base:base + HD,
                                         q0 + qc * QC:q0 + (qc + 1) * QC],
                                start=True,
                                stop=True,
                            )
                        if ks == 0 and carry is not None:
                            emit_carry(carry)
                            carry = None
                        at = apool.tile([128, QG], BF16, tag="at")
                        nc.scalar.activation(
                            at, psS, mybir.ActivationFunctionType.Exp,
                            scale=0.125,
                        )
                        nc.vector.tensor_mul(at, at, nm_sb[:, ks, q0:q0 + QG])
                        ats[ks] = at
                    if psO is None:
                        psO = psO_pool.tile([128, NCH, 128], F32)
                    tail = sorted(ats.items())
                    carry = (h, qg, psO, [(a, k) for k, a in tail])
                emit_carry(carry)

            for _ in range(reps):
                emit_body()
    nc.compile()
    return nc


_CACHE = {}


def _get_nc():
    if "nc" not in _CACHE:
        _CACHE["nc"] = build_program()
    return _CACHE["nc"]


def make_in_maps(q, k, v, mask, s=S):
    """Shard full inputs into 8 per-core input maps (host-side layout prep)."""
    q = np.asarray(q, dtype=np.float32)
    k = np.asarray(k, dtype=np.float32)
    v = np.asarray(v, dtype=np.float32)
    mask = np.asarray(mask)
    nh = q.shape[-1] // HD
    in_maps = []
    for c in range(NCORES):
        b, g = divmod(c, NCORES // B)
        h0 = HPC * g
        qs = q[b].reshape(s, nh, HD)[:, h0:h0 + HPC, :]      # [s, HPC, 64]
        ks_ = k[b].reshape(s, nh, HD)[:, h0:h0 + HPC, :]
        qkT = np.empty((2, HPC * HD, s), ml_dtypes.bfloat16)
        qkT[0] = qs.transpose(1, 2, 0).reshape(HPC * HD, s)
        qkT[1] = ks_.transpose(1, 2, 0).reshape(HPC * HD, s)
        vc = np.ascontiguousarray(v[b, :, h0 * HD:(h0 + HPC) * HD]).astype(
            ml_dtypes.bfloat16
        )
        nmT = np.ascontiguousarray((~mask[b]).T).astype(ml_dtypes.bfloat16)
        in_maps.append({"qkT": qkT, "v": vc, "nmT": nmT})
    return in_maps


def assemble_out(results, s=S, d=D):
    out = np.empty((B, s, d), np.float32)
    for c in range(NCORES):
        b, g = divmod(c, NCORES // B)
        out[b, :, g * HPC * HD:(g + 1) * HPC * HD] = results[c]["out"]
    return out


def kernel(q, k, v, mask):
    nc = _get_nc()
    in_maps = make_in_maps(q, k, v, mask)
    res = run_bass_kernel_spmd(nc, in_maps, list(range(NCORES))).results
    return assemble_out(res)


# revision 13
# speedup vs baseline: 1.0703x; 1.0609x over previous
# Multi-head attention (B=2, S=2048, D=1024, H=16, head_dim=64) with bool mask,
# sharded across 8 TRN2 NeuronCores: core c -> batch c//4, heads 4*(c%4)..4*(c%4)+3.
#
# Per-core device kernel (scores computed transposed: scoresT[k, q]):
#   scoresT = K @ Q^T                 (PE bf16, lhsT = K^T strip, rhs = Q^T)
#   atp     = exp(scoresT/8)          (ACT exp scale=1/8, psum -> psum bf16)
#   at      = atp * (1-m)T            (DVE mult, psum -> SBUF bf16)
#   out[q,d] += at_chunk^T @ [V|1]    (PE bf16: lhsT = at chunk (stationary),
#                                      rhs = V'[128,65]; col 64 accumulates Z)
#   out     = psO[:, :, 0:64] / Z     (DVE reciprocal + broadcast multiply)
#
# The AV matmul uses the attention chunk as the stationary operand so the
# output lands non-transposed ([q, d] with q on partitions): free size is 65
# instead of 512 per instruction (half the PE cycles of the V-stationary
# form) and the final PE transposes disappear entirely.
#
# Host side (inside kernel()): slice per-core shards, pre-transpose Q/K per
# head ([64, S] head-dim-major, bf16), pre-transpose the inverted mask to
# bf16, reassemble the 8 per-core bf16 outputs into the full f32 output.

import sys

import numpy as np

for _p in ("/opt/trn_rl_repo",):
    if _p not in sys.path:
        sys.path.insert(0, _p)

import ml_dtypes

import concourse.bass as bass  # noqa: F401  (engine types reachable via nc)
import concourse.tile as tile
from concourse import bacc, mybir
from concourse.bass_utils import run_bass_kernel_spmd

F32 = mybir.dt.float32
BF16 = mybir.dt.bfloat16

S = 2048          # sequence length
HD = 64           # head dim
HPC = 4           # heads per core
NCORES = 8
B = 2
H = 16
D = H * HD


def build_program(s=S, reps=1):
    """Build the single-core SPMD program. Returns the compiled Bacc object.

    reps>1 emits the whole body (loads+compute+stores) that many times in one
    NEFF — used to measure device time by wall-clock differencing."""
    nc = bacc.Bacc()

    KS = s // 128            # number of k strips
    QG = 1024 if s >= 1024 else s   # q group width (ACT/DVE instruction width)
    NQG = s // QG            # q groups
    NQC = max(QG // 512, 1)  # 512-wide matmul chunks per q group (psum bank)
    QC = min(512, QG)        # matmul chunk width
    NCH = QG // 128          # 128-wide q chunks per group (AV granularity)
    CPB = 4                  # psO chunks per 2KB psum bank (zero region)
    LAG = min(4, KS)         # AV strips emitted this many strips behind QK

    qkT_d = nc.declare_dram_parameter("qkT", [2, HPC * HD, s], BF16, isOutput=False)
    v_d = nc.declare_dram_parameter("v", [s, HPC * HD], BF16, isOutput=False)
    nmT_d = nc.declare_dram_parameter("nmT", [s, s], BF16, isOutput=False)
    out_d = nc.declare_dram_parameter("out", [s, HPC * HD], BF16, isOutput=True)

    # DRAM views with the k/q axis split into strips of 128 partitions
    nm_view = nmT_d[:].rearrange("(ks p) q -> p ks q", p=128)
    v_view = v_d[:].rearrange("(ks p) (h d) -> p ks h d", p=128, h=HPC)
    out_view = out_d[:].rearrange("(sq p) c -> p sq c", p=128)

    with tile.TileContext(nc) as tc:
        with (
            tc.tile_pool(name="const", bufs=1) as const,
            tc.tile_pool(name="wq", bufs=1) as wq,
            tc.tile_pool(name="attn", bufs=20) as apool,
            tc.tile_pool(name="stat", bufs=4) as spool,
            tc.tile_pool(name="oasm", bufs=1) as opool,
            tc.tile_pool(name="psS", bufs=2, space="PSUM") as psS_pool,
            tc.tile_pool(name="psO", bufs=2, space="PSUM") as psO_pool,
        ):
            # Preload the exp table (emitted before any real exp; runs while
            # the first DMAs stream).
            warm = const.tile([128, 1], F32)
            nc.vector.memset(warm, 0.0)
            nc.scalar.activation(warm, warm, mybir.ActivationFunctionType.Exp)

            # Warm the PE HAM clock gate while input DMAs run: ~3us of dummy
            # matmuls so the first real QKs run at 2.4GHz.
            zb = const.tile([128, 128], BF16)
            nc.vector.memset(zb, 0.0)
            for _ in range(24):
                wmm = psS_pool.tile([128, QG], F32, tag="psS")
                nc.tensor.matmul(
                    wmm[:, :128], lhsT=zb[0:64, :], rhs=zb[0:64, :],
                    start=True, stop=True,
                )

            def qk_src(pair):
                return qkT_d[:, 128 * pair:128 * pair + 128, :].rearrange(
                    "t p s -> p t s"
                )

            def emit_body():
                # Q^T / K^T head pairs: [128, s] (head 2p on partitions 0-63,
                # head 2p+1 on partitions 64-127).
                qks = []
                for pair in range(HPC // 2):
                    qk = wq.tile([128, 2, s], BF16, tag=f"qkT{pair}")
                    qks.append(qk)
                # V' staging: [128, ks, h, 65] with a ones column at 64 so the
                # AV matmul's 65th output column accumulates the softmax
                # denominator Z. V lands via interleaved DMA; the ones column
                # is memset once (disjoint subtile, no dependency on the DMA).
                vps = wq.tile([128, KS, HPC, HD + 1], BF16, tag="vps")
                nm_sb = wq.tile([128, KS, s], BF16, tag="nm")
                nc.vector.memset(vps[:, :, :, HD:HD + 1], 1.0)

                # DMA choreography (s=2048): two queues only — SP (nc.sync)
                # and Pool SWDGE (nc.gpsimd) — so the ACT and DVE sequencers
                # are never blocked behind a DMA wait. Ordered by first use:
                # K strips + first Q group first, mask halves interleaved,
                # V early (AV matmuls sit in the in-order PE queue).
                QH = QG  # nm half width
                if s == 2048:
                    A, Bq = nc.sync, nc.gpsimd
                    # The model's DMA device is effectively serial, so the
                    # ordering across all three queues is what matters: the
                    # first-QK inputs, then mask halves at the consumption
                    # rate, with V and the second head-pair deferred to their
                    # first use. The ACT queue carries only the two DMAs that
                    # gate the first exp (it is idle until then).
                    nc.scalar.dma_start(out=qks[0][:, 1, 0:512], in_=qk_src(0)[:, 1, 0:512])
                    nc.scalar.dma_start(out=qks[0][:, 0, 0:QG], in_=qk_src(0)[:, 0, 0:QG])
                    A.dma_start(out=qks[0][:, 1, 512:1024], in_=qk_src(0)[:, 1, 512:1024])
                    Bq.dma_start(out=vps[:, :, 0, 0:HD], in_=v_view[:, :, 0])
                    A.dma_start(out=nm_sb[:, 0, 0:QH], in_=nm_view[:, 0, 0:QH])
                    Bq.dma_start(out=nm_sb[:, 1, 0:QH], in_=nm_view[:, 1, 0:QH])
                    A.dma_start(out=nm_sb[:, 2, 0:QH], in_=nm_view[:, 2, 0:QH])
                    A.dma_start(out=qks[0][:, 1, 1024:2048], in_=qk_src(0)[:, 1, 1024:2048])
                    Bq.dma_start(out=nm_sb[:, 3, 0:QH], in_=nm_view[:, 3, 0:QH])
                    for ks in range(4, KS):
                        (A if ks % 2 == 0 else Bq).dma_start(
                            out=nm_sb[:, ks, 0:QH], in_=nm_view[:, ks, 0:QH]
                        )
                        if ks == 8:
                            A.dma_start(out=qks[0][:, 0, QG:2 * QG],
                                        in_=qk_src(0)[:, 0, QG:2 * QG])
                    # second batch: V heads 1-3, q-group-1 mask halves, pair 1
                    Bq.dma_start(out=vps[:, :, 1, 0:HD], in_=v_view[:, :, 1])
                    for ks in range(KS):
                        (A if ks % 2 == 0 else Bq).dma_start(
                            out=nm_sb[:, ks, QH:2 * QH], in_=nm_view[:, ks, QH:2 * QH]
                        )
                        if ks == 6:
                            Bq.dma_start(out=vps[:, :, 2, 0:HD], in_=v_view[:, :, 2])
                        if ks == 10:
                            A.dma_start(out=qks[1], in_=qk_src(1))
                        if ks == 12:
                            Bq.dma_start(out=vps[:, :, 3, 0:HD], in_=v_view[:, :, 3])
                else:
                    A, Bq = nc.sync, nc.gpsimd
                    A.dma_start(out=qks[0], in_=qk_src(0))
                    for hh in range(HPC):
                        Bq.dma_start(out=vps[:, :, hh, 0:HD], in_=v_view[:, :, hh])
                    for pair in range(1, HPC // 2):
                        A.dma_start(out=qks[pair], in_=qk_src(pair))
                    for ks in range(KS):
                        (A if ks % 2 == 0 else Bq).dma_start(
                            out=nm_sb[:, ks, :], in_=nm_view[:, ks, :]
                        )

                out_asm = opool.tile([128, KS, HPC * HD], BF16)

                def emit_carry(carry):
                    """Last two AV strips (lag-2 emission) + finalize: Z
                    reciprocal, broadcast normalize, and the output DMA once
                    the last head of a q-group completes."""
                    ch, cqg, cpsO, at_tail = carry
                    for i, (cat, cks) in enumerate(at_tail):
                        last = i == len(at_tail) - 1
                        for j in range(NCH):
                            nc.tensor.matmul(
                                cpsO[:, j, 0:HD + 1],
                                lhsT=cat[:, j * 128:(j + 1) * 128],
                                rhs=vps[:, cks, ch, :],
                                # start/stop once per psum bank (4 chunks of
                                # 512B share a 2KB zero region)
                                start=(cks == 0 and j % CPB == 0),
                                stop=last and (j % CPB == CPB - 1 or j == NCH - 1),
                            )
                    rec = spool.tile([128, NCH], F32)
                    nc.vector.reciprocal(rec, cpsO[:, :, HD])
                    # last head of a q-group: finer chunks so output DMAs
                    # start earlier (the final one is on the critical path)
                    step = max(NCH // (4 if ch == HPC - 1 else 2), 1)
                    for lo in range(0, NCH, step):
                        hi = min(lo + step, NCH)
                        sq0 = cqg * NCH + lo
                        nc.vector.tensor_mul(
                            out_asm[:, sq0:sq0 + hi - lo, ch * HD:(ch + 1) * HD],
                            cpsO[:, lo:hi, 0:HD],
                            rec[:, lo:hi].to_broadcast([128, hi - lo, HD]),
                        )
                        if ch == HPC - 1:
                            eng = nc.sync if (lo // step) % 2 == 0 else nc.gpsimd
                            eng.dma_start(
                                out=out_view[:, sq0:sq0 + hi - lo, :],
                                in_=out_asm[:, sq0:sq0 + hi - lo, :],
                            )

                carry = None
                groups = [(h, qg) for h in range(HPC) for qg in range(NQG)]
                for h, qg in groups:
                    base = 64 * (h % 2)
                    qt_r = qks[h // 2][:, 0, :]
                    kt_r = qks[h // 2][:, 1, :]
                    q0 = qg * QG
                    psO = None
                    ats = {}
                    for ks in range(KS):
                        # AV emitted with a lag of LAG strips: a late mask
                        # multiply (nm DMA still in flight) then never blocks
                        # the in-order PE queue right before the QK the ACT
                        # engine is waiting on.
                        if ks - LAG in ats:
                            at2 = ats.pop(ks - LAG)
                            if psO is None:
                                psO = psO_pool.tile([128, NCH, 128], F32)
                            for j in range(NCH):
                                nc.tensor.matmul(
                                    psO[:, j, 0:HD + 1],
                                    lhsT=at2[:, j * 128:(j + 1) * 128],
                                    rhs=vps[:, ks - LAG, h, :],
                                    start=(ks == LAG and j % CPB == 0),
                                    stop=False,
                                )
                        psS = psS_pool.tile([128, QG], F32, tag="psS")
                        for qc in range(NQC):
                            nc.tensor.matmul(
                                psS[:, qc * QC:(qc + 1) * QC],
                                lhsT=kt_r[base:base + HD, ks * 128:(ks + 1) * 128],
                                rhs=qt_r[base:base + HD,
                                         q0 + qc * QC:q0 + (qc + 1) * QC],
                                start=True,
                                stop=True,
                            )
                        if ks == 0 and carry is not None:
                            emit_carry(carry)
                            carry = None
                        at = apool.tile([128, QG], BF16, tag="at")
                        nc.scalar.activation(
                            at, psS, mybir.ActivationFunctionType.Exp,
                            scale=0.125,
                        )
                        nc.vector.tensor_mul(at, at, nm_sb[:, ks, q0:q0 + QG])
                        ats[ks] = at
                    if psO is None:
                        psO = psO_pool.tile([128, NCH, 128], F32)
                    tail = sorted(ats.items())
                    carry = (h, qg, psO, [(a, k) for k, a in tail])
                emit_carry(carry)

            for _ in range(reps):
                emit_body()
    nc.compile()
    return nc


_CACHE = {}


def _get_nc():
    if "nc" not in _CACHE:
        _CACHE["nc"] = build_program()
    return _CACHE["nc"]


def make_in_maps(q, k, v, mask, s=S):
    """Shard full inputs into 8 per-core input maps (host-side layout prep)."""
    q = np.asarray(q, dtype=np.float32)
    k = np.asarray(k, dtype=np.float32)
    v = np.asarray(v, dtype=np.float32)
    mask = np.asarray(mask)
    nh = q.shape[-1] // HD
    in_maps = []
    for c in range(NCORES):
        b, g = divmod(c, NCORES // B)
        h0 = HPC * g
        qs = q[b].reshape(s, nh, HD)[:, h0:h0 + HPC, :]      # [s, HPC, 64]
        ks_ = k[b].reshape(s, nh, HD)[:, h0:h0 + HPC, :]
        qkT = np.empty((2, HPC * HD, s), ml_dtypes.bfloat16)
        qkT[0] = qs.transpose(1, 2, 0).reshape(HPC * HD, s)
        qkT[1] = ks_.transpose(1, 2, 0).reshape(HPC * HD, s)
        vc = np.ascontiguousarray(v[b, :, h0 * HD:(h0 + HPC) * HD]).astype(
            ml_dtypes.bfloat16
        )
        nmT = np.ascontiguousarray((~mask[b]).T).astype(ml_dtypes.bfloat16)
        in_maps.append({"qkT": qkT, "v": vc, "nmT": nmT})
    return in_maps


def assemble_out(results, s=S, d=D):
    out = np.empty((B, s, d), np.float32)
    for c in range(NCORES):
        b, g = divmod(c, NCORES // B)
        out[b, :, g * HPC * HD:(g + 1) * HPC * HD] = results[c]["out"]
    return out


def kernel(q, k, v, mask):
    nc = _get_nc()
    in_maps = make_in_maps(q, k, v, mask)
    res = run_bass_kernel_spmd(nc, in_maps, list(range(NCORES))).results
    return assemble_out(res)


# revision 16
# speedup vs baseline: 1.0806x; 1.0096x over previous
# Multi-head attention (B=2, S=2048, D=1024, H=16, head_dim=64) with bool mask,
# sharded across 8 TRN2 NeuronCores: core c -> batch c//4, heads 4*(c%4)..4*(c%4)+3.
#
# Per-core device kernel (scores computed transposed: scoresT[k, q]):
#   scoresT = K @ Q^T                 (PE bf16, lhsT = K^T strip, rhs = Q^T)
#   atp     = exp(scoresT/8)          (ACT exp scale=1/8, psum -> psum bf16)
#   at      = atp * (1-m)T            (DVE mult, psum -> SBUF bf16)
#   out[q,d] += at_chunk^T @ [V|1]    (PE bf16: lhsT = at chunk (stationary),
#                                      rhs = V'[128,65]; col 64 accumulates Z)
#   out     = psO[:, :, 0:64] / Z     (DVE reciprocal + broadcast multiply)
#
# The AV matmul uses the attention chunk as the stationary operand so the
# output lands non-transposed ([q, d] with q on partitions): free size is 65
# instead of 512 per instruction (half the PE cycles of the V-stationary
# form) and the final PE transposes disappear entirely.
#
# Host side (inside kernel()): slice per-core shards, pre-transpose Q/K per
# head ([64, S] head-dim-major, bf16), pre-transpose the inverted mask to
# bf16, reassemble the 8 per-core bf16 outputs into the full f32 output.

import sys

import numpy as np

for _p in ("/opt/trn_rl_repo",):
    if _p not in sys.path:
        sys.path.insert(0, _p)

import ml_dtypes

import concourse.bass as bass  # noqa: F401  (engine types reachable via nc)
import concourse.tile as tile
from concourse import bacc, mybir
from concourse.bass_utils import run_bass_kernel_spmd

F32 = mybir.dt.float32
BF16 = mybir.dt.bfloat16

S = 2048          # sequence length
HD = 64           # head dim
HPC = 4           # heads per core
NCORES = 8
B = 2
H = 16
D = H * HD


def build_program(s=S, reps=1):
    """Build the single-core SPMD program. Returns the compiled Bacc object.

    reps>1 emits the whole body (loads+compute+stores) that many times in one
    NEFF — used to measure device time by wall-clock differencing."""
    nc = bacc.Bacc()

    KS = s // 128            # number of k strips
    QG = 1024 if s >= 1024 else s   # q group width (ACT/DVE instruction width)
    NQG = s // QG            # q groups
    NQC = max(QG // 512, 1)  # 512-wide matmul chunks per q group (psum bank)
    QC = min(512, QG)        # matmul chunk width
    NCH = QG // 128          # 128-wide q chunks per group (AV granularity)
    CPB = 4                  # psO chunks per 2KB psum bank (zero region)
    LAG = min(4, KS)         # AV strips emitted this many strips behind QK

    qkT_d = nc.declare_dram_parameter("qkT", [2, HPC * HD, s], BF16, isOutput=False)
    v_d = nc.declare_dram_parameter("v", [s, HPC * HD], BF16, isOutput=False)
    nmT_d = nc.declare_dram_parameter("nmT", [s, s], BF16, isOutput=False)
    out_d = nc.declare_dram_parameter("out", [s, HPC * HD], BF16, isOutput=True)

    # DRAM views with the k/q axis split into strips of 128 partitions
    nm_view = nmT_d[:].rearrange("(ks p) q -> p ks q", p=128)
    v_view = v_d[:].rearrange("(ks p) (h d) -> p ks h d", p=128, h=HPC)
    out_view = out_d[:].rearrange("(sq p) c -> p sq c", p=128)

    with tile.TileContext(nc) as tc:
        with (
            tc.tile_pool(name="const", bufs=1) as const,
            tc.tile_pool(name="wq", bufs=1) as wq,
            tc.tile_pool(name="attn", bufs=20) as apool,
            tc.tile_pool(name="stat", bufs=4) as spool,
            tc.tile_pool(name="oasm", bufs=1) as opool,
            tc.tile_pool(name="psS", bufs=2, space="PSUM") as psS_pool,
            tc.tile_pool(name="psO", bufs=2, space="PSUM") as psO_pool,
        ):
            # Preload the exp table (emitted before any real exp; runs while
            # the first DMAs stream).
            warm = const.tile([128, 1], F32)
            nc.vector.memset(warm, 0.0)
            nc.scalar.activation(warm, warm, mybir.ActivationFunctionType.Exp)

            # Warm the PE HAM clock gate while input DMAs run: ~3us of dummy
            # matmuls so the first real QKs run at 2.4GHz.
            zb = const.tile([128, 128], BF16)
            nc.vector.memset(zb, 0.0)
            for _ in range(24):
                wmm = psS_pool.tile([128, QG], F32, tag="psS")
                nc.tensor.matmul(
                    wmm[:, :128], lhsT=zb[0:64, :], rhs=zb[0:64, :],
                    start=True, stop=True,
                )

            def qk_src(pair):
                return qkT_d[:, 128 * pair:128 * pair + 128, :].rearrange(
                    "t p s -> p t s"
                )

            def emit_body():
                # Q^T / K^T head pairs: [128, s] (head 2p on partitions 0-63,
                # head 2p+1 on partitions 64-127).
                qks = []
                for pair in range(HPC // 2):
                    qk = wq.tile([128, 2, s], BF16, tag=f"qkT{pair}")
                    qks.append(qk)
                # V' staging: [128, ks, h, 65] with a ones column at 64 so the
                # AV matmul's 65th output column accumulates the softmax
                # denominator Z. V lands via interleaved DMA; the ones column
                # is memset once (disjoint subtile, no dependency on the DMA).
                vps = wq.tile([128, KS, HPC, HD + 1], BF16, tag="vps")
                nm_sb = wq.tile([128, KS, s], BF16, tag="nm")
                nc.vector.memset(vps[:, :, :, HD:HD + 1], 1.0)

                # DMA choreography (s=2048): two queues only — SP (nc.sync)
                # and Pool SWDGE (nc.gpsimd) — so the ACT and DVE sequencers
                # are never blocked behind a DMA wait. Ordered by first use:
                # K strips + first Q group first, mask halves interleaved,
                # V early (AV matmuls sit in the in-order PE queue).
                QH = QG  # nm half width
                if s == 2048:
                    A, Bq = nc.sync, nc.gpsimd
                    # The model's DMA device is effectively serial, so the
                    # ordering across the queues is what matters: the
                    # first-QK inputs lead on SP (issued at t=0; the ACT
                    # queue is busy with the exp-table warmup), then mask
                    # halves at the consumption rate, with V and the second
                    # head-pair deferred to their first use.
                    A.dma_start(out=qks[0][:, 1, 0:512], in_=qk_src(0)[:, 1, 0:512])
                    A.dma_start(out=qks[0][:, 0, 0:QG], in_=qk_src(0)[:, 0, 0:QG])
                    A.dma_start(out=qks[0][:, 1, 512:1024], in_=qk_src(0)[:, 1, 512:1024])
                    Bq.dma_start(out=vps[:, :, 0, 0:HD], in_=v_view[:, :, 0])
                    A.dma_start(out=nm_sb[:, 0, 0:QH], in_=nm_view[:, 0, 0:QH])
                    Bq.dma_start(out=nm_sb[:, 1, 0:QH], in_=nm_view[:, 1, 0:QH])
                    A.dma_start(out=nm_sb[:, 2, 0:QH], in_=nm_view[:, 2, 0:QH])
                    A.dma_start(out=qks[0][:, 1, 1024:2048], in_=qk_src(0)[:, 1, 1024:2048])
                    Bq.dma_start(out=nm_sb[:, 3, 0:QH], in_=nm_view[:, 3, 0:QH])
                    for ks in range(4, KS):
                        (A if ks % 2 == 0 else Bq).dma_start(
                            out=nm_sb[:, ks, 0:QH], in_=nm_view[:, ks, 0:QH]
                        )
                        if ks == 8:
                            A.dma_start(out=qks[0][:, 0, QG:2 * QG],
                                        in_=qk_src(0)[:, 0, QG:2 * QG])
                    # second batch: V heads 1-3, q-group-1 mask halves, pair 1
                    Bq.dma_start(out=vps[:, :, 1, 0:HD], in_=v_view[:, :, 1])
                    for ks in range(KS):
                        (A if ks % 2 == 0 else Bq).dma_start(
                            out=nm_sb[:, ks, QH:2 * QH], in_=nm_view[:, ks, QH:2 * QH]
                        )
                        if ks == 6:
                            Bq.dma_start(out=vps[:, :, 2, 0:HD], in_=v_view[:, :, 2])
                        if ks == 10:
                            A.dma_start(out=qks[1], in_=qk_src(1))
                        if ks == 12:
                            Bq.dma_start(out=vps[:, :, 3, 0:HD], in_=v_view[:, :, 3])
                else:
                    A, Bq = nc.sync, nc.gpsimd
                    A.dma_start(out=qks[0], in_=qk_src(0))
                    for hh in range(HPC):
                        Bq.dma_start(out=vps[:, :, hh, 0:HD], in_=v_view[:, :, hh])
                    for pair in range(1, HPC // 2):
                        A.dma_start(out=qks[pair], in_=qk_src(pair))
                    for ks in range(KS):
                        (A if ks % 2 == 0 else Bq).dma_start(
                            out=nm_sb[:, ks, :], in_=nm_view[:, ks, :]
                        )

                out_asm = opool.tile([128, KS, HPC * HD], BF16)

                def emit_carry(carry):
                    """Last two AV strips (lag-2 emission) + finalize: Z
                    reciprocal, broadcast normalize, and the output DMA once
                    the last head of a q-group completes."""
                    ch, cqg, cpsO, at_tail = carry
                    for i, (cat, cks) in enumerate(at_tail):
                        last = i == len(at_tail) - 1
                        for j in range(NCH):
                            nc.tensor.matmul(
                                cpsO[:, j, 0:HD + 1],
                                lhsT=cat[:, j * 128:(j + 1) * 128],
                                rhs=vps[:, cks, ch, :],
                                # start/stop once per psum bank (4 chunks of
                                # 512B share a 2KB zero region)
                                start=(cks == 0 and j % CPB == 0),
                                stop=last and (j % CPB == CPB - 1 or j == NCH - 1),
                            )
                    rec = spool.tile([128, NCH], F32)
                    nc.vector.reciprocal(rec, cpsO[:, :, HD])
                    final = ch == HPC - 1 and cqg == NQG - 1
                    step = max(NCH // 2, 1)
                    for lo in range(0, NCH, step):
                        hi = min(lo + step, NCH)
                        sq0 = cqg * NCH + lo
                        nc.vector.tensor_mul(
                            out_asm[:, sq0:sq0 + hi - lo, ch * HD:(ch + 1) * HD],
                            cpsO[:, lo:hi, 0:HD],
                            rec[:, lo:hi].to_broadcast([128, hi - lo, HD]),
                        )
                        if ch == HPC - 1:
                            # the final group's DMAs both ride SP: the Pool
                            # SWDGE prep (~1.1us) would sit on the critical
                            # path at the very end of the program
                            eng = nc.gpsimd if (lo > 0 and not final) else nc.sync
                            eng.dma_start(
                                out=out_view[:, sq0:sq0 + hi - lo, :],
                                in_=out_asm[:, sq0:sq0 + hi - lo, :],
                            )

                carry = None
                groups = [(h, qg) for h in range(HPC) for qg in range(NQG)]
                for gi, (h, qg) in enumerate(groups):
                    base = 64 * (h % 2)
                    qt_r = qks[h // 2][:, 0, :]
                    kt_r = qks[h // 2][:, 1, :]
                    q0 = qg * QG
                    # AV lag: during the DMA-racy head a late mask multiply
                    # must not block the in-order PE queue right before the
                    # QK the ACT engine is waiting on; later groups use lag 1
                    # so the end-of-group AV backlog (and the final tail) is
                    # short.
                    lag = LAG if gi < 2 else 1
                    last_g = gi == len(groups) - 1
                    psO = None
                    ats = {}
                    for ks in range(KS):
                        if ks - lag in ats:
                            at2 = ats.pop(ks - lag)
                            if psO is None:
                                psO = psO_pool.tile([128, NCH, 128], F32)
                            for j in range(NCH):
                                nc.tensor.matmul(
                                    psO[:, j, 0:HD + 1],
                                    lhsT=at2[:, j * 128:(j + 1) * 128],
                                    rhs=vps[:, ks - lag, h, :],
                                    start=(ks == lag and j % CPB == 0),
                                    stop=False,
                                )
                        psS = psS_pool.tile([128, QG], F32, tag="psS")
                        for qc in range(NQC):
                            nc.tensor.matmul(
                                psS[:, qc * QC:(qc + 1) * QC],
                                lhsT=kt_r[base:base + HD, ks * 128:(ks + 1) * 128],
                                rhs=qt_r[base:base + HD,
                                         q0 + qc * QC:q0 + (qc + 1) * QC],
                                start=True,
                                stop=True,
                            )
                        if ks == 0 and carry is not None:
                            emit_carry(carry)
                            carry = None
                        at = apool.tile([128, QG], BF16, tag="at")
                        # final strip of the final group: exp+mask in halves
                        # so the tail AV chunks start half an exp earlier
                        # (subtile deps let AV chunks 0-3 run off half 1)
                        nsp = 2 if (last_g and ks == KS - 1 and QG >= 1024) else 1
                        for sp in range(nsp):
                            sl = slice(sp * QG // nsp, (sp + 1) * QG // nsp)
                            nc.scalar.activation(
                                at[:, sl], psS[:, sl],
                                mybir.ActivationFunctionType.Exp,
                                scale=0.125,
                            )
                            nc.vector.tensor_mul(
                                at[:, sl], at[:, sl],
                                nm_sb[:, ks, q0 + sl.start:q0 + sl.stop],
                            )
                        ats[ks] = at
                    if psO is None:
                        psO = psO_pool.tile([128, NCH, 128], F32)
                    tail = sorted(ats.items())
                    carry = (h, qg, psO, [(a, k) for k, a in tail])
                emit_carry(carry)

            for _ in range(reps):
                emit_body()
    nc.compile()
    return nc


_CACHE = {}


def _get_nc():
    if "nc" not in _CACHE:
        _CACHE["nc"] = build_program()
    return _CACHE["nc"]


def make_in_maps(q, k, v, mask, s=S):
    """Shard full inputs into 8 per-core input maps (host-side layout prep)."""
    q = np.asarray(q, dtype=np.float32)
    k = np.asarray(k, dtype=np.float32)
    v = np.asarray(v, dtype=np.float32)
    mask = np.asarray(mask)
    nh = q.shape[-1] // HD
    in_maps = []
    for c in range(NCORES):
        b, g = divmod(c, NCORES // B)
        h0 = HPC * g
        qs = q[b].reshape(s, nh, HD)[:, h0:h0 + HPC, :]      # [s, HPC, 64]
        ks_ = k[b].reshape(s, nh, HD)[:, h0:h0 + HPC, :]
        qkT = np.empty((2, HPC * HD, s), ml_dtypes.bfloat16)
        qkT[0] = qs.transpose(1, 2, 0).reshape(HPC * HD, s)
        qkT[1] = ks_.transpose(1, 2, 0).reshape(HPC * HD, s)
        vc = np.ascontiguousarray(v[b, :, h0 * HD:(h0 + HPC) * HD]).astype(
            ml_dtypes.bfloat16
        )
        nmT = np.ascontiguousarray((~mask[b]).T).astype(ml_dtypes.bfloat16)
        in_maps.append({"qkT": qkT, "v": vc, "nmT": nmT})
    return in_maps


def assemble_out(results, s=S, d=D):
    out = np.empty((B, s, d), np.float32)
    for c in range(NCORES):
        b, g = divmod(c, NCORES // B)
        out[b, :, g * HPC * HD:(g + 1) * HPC * HD] = results[c]["out"]
    return out


def kernel(q, k, v, mask):
    nc = _get_nc()
    in_maps = make_in_maps(q, k, v, mask)
    res = run_bass_kernel_spmd(nc, in_maps, list(range(NCORES))).results
    return assemble_out(res)
